# revision 1
# baseline (speedup 1.0000x reference)
"""Trainium2 Bass kernel for nn_KL_PS_Loss (PowerSpherical contrastive KL loss).

Host side: replicates the reference's MC sampling (jax CPU, key 42) exactly,
shards the n_mc=1000 sample axis across 8 NeuronCores.
Device side (per core, 125 samples): m = loc @ mc_n^T via bf16 matmuls into
f32 PSUM, then a running elementwise product of (1+m) — split between a
DVE scalar_tensor_tensor path (f32, fused (psum+1)*prod, 1x) and an
ACT-evacuate path (Identity+bias -> bf16 SBUF, then DVE tensor_mul at 2x) to
balance ACT/DVE load.  ln(prod) once at the end recovers sum(log1p(m)).
Host gathers the 8 partial sums, means, and runs the tiny (512,512)
contrastive step in f64.
"""

import os
import numpy as np

# ---- problem constants (hardcoded; must match reference.py) ----
N_MC = 1000
N2, D = 512, 128          # 2n, d
TEMPERATURE = 0.1
DIAG_FILL = -9e15
_EPS = 1e-12
N_CORES = 8
S_PER_CORE = N_MC // N_CORES   # 125
DMA_GROUP = 5                  # samples per input DMA
N_GROUPS = S_PER_CORE // DMA_GROUP  # 25
STT_EVERY = 4                  # kept for test_sim compatibility
PATTERN = ["S", "D", "G", "S", "D", "G", "S", "G"]  # per-sample path, cycles mod 8

_CACHE = {}
LAST_INFO = {}


def _host_samples(loc_np: np.ndarray, scale_np: np.ndarray) -> np.ndarray:
    """Exact replica of reference._rsample(jax.random.key(42), loc, scale, N_MC)
    on the jax CPU backend.  Returns (N_MC, 2n, d) float32."""
    import jax
    import jax.numpy as jnp

    cpu = jax.devices("cpu")[0]
    with jax.default_device(cpu):
        loc = jnp.asarray(loc_np, jnp.float32)
        scale = jnp.asarray(scale_np, jnp.float32)
        n, d = loc.shape
        b = (d - 1) / 2.0
        a = b + scale
        key = jax.random.key(42)
        k1, k2 = jax.random.split(key)
        z = jax.random.beta(k1, a, b, shape=(N_MC, n))
        t = 2.0 * z - 1.0
        v = jax.random.normal(k2, (N_MC, n, d - 1))
        v = v / jnp.linalg.norm(v, axis=-1, keepdims=True)
        t_ = t[..., None]
        y = jnp.concatenate(
            [t_, v * jnp.sqrt(jnp.clip(1.0 - t_ * t_, 1e-20))], axis=-1
        )
        e1 = jnp.zeros((n, d), loc.dtype).at[:, 0].set(1.0)
        u = e1 - loc
        u = u / (jnp.linalg.norm(u, axis=-1, keepdims=True) + _EPS)
        mc = y - 2.0 * jnp.sum(y * u, axis=-1, keepdims=True) * u
        return np.asarray(mc, dtype=np.float32)


def _build_nc():
    """Build the per-core Bass/Tile program (SPMD; same NEFF on all cores)."""
    import concourse.tile as tile
    from concourse import bacc, mybir

    f32 = mybir.dt.float32
    bf16 = mybir.dt.bfloat16
    AF = mybir.ActivationFunctionType

    nc = bacc.Bacc()
    locT = nc.dram_tensor("locT", (128, N2), bf16, kind="ExternalInput")
    mc = nc.dram_tensor(
        "mc", (N_GROUPS, 128, DMA_GROUP * N2), bf16, kind="ExternalInput"
    )
    out = nc.dram_tensor("out", (3, 128, 4 * N2), f32, kind="ExternalOutput")

    with tile.TileContext(nc) as tc:
        with (
            tc.tile_pool(name="const", bufs=1) as const_pool,
            tc.tile_pool(name="mcp", bufs=3) as mc_pool,
            tc.tile_pool(name="tmpp", bufs=4) as tmp_pool,
            tc.tile_pool(name="persist", bufs=1) as persist,
            tc.tile_pool(name="ps", bufs=2, space="PSUM") as ps_pool,
        ):
            locT_sb = const_pool.tile([128, N2], bf16)
            nc.sync.dma_start(out=locT_sb, in_=locT[:, :])

            # running products: prod_f (f32, fused-STT on DVE straight from
            # PSUM), prod_b (bf16, DVE mult), prod_g (bf16, GPSIMD mult).
            prod_f = persist.tile([128, 4 * N2], f32)
            prod_b = persist.tile([128, 4 * N2], bf16)
            prod_g = persist.tile([128, 4 * N2], bf16)

            first = {"D": True, "G": True}
            for g in range(N_GROUPS):
                mc_sb = mc_pool.tile([128, DMA_GROUP * N2], bf16)
                nc.sync.dma_start(out=mc_sb, in_=mc[g])
                for r in range(DMA_GROUP):
                    s = g * DMA_GROUP + r
                    rhs = mc_sb[:, r * N2 : (r + 1) * N2]
                    ps = ps_pool.tile([128, 4 * N2], f32)
                    for blk in range(4):
                        nc.tensor.matmul(
                            ps[:, blk * N2 : (blk + 1) * N2],
                            locT_sb[:, blk * 128 : (blk + 1) * 128],
                            rhs,
                            start=True,
                            stop=True,
                        )
                    path = PATTERN[s % len(PATTERN)]
                    if path == "S":
                        if s == 0:
                            nc.vector.tensor_scalar_add(prod_f, ps, 1.0)
                        else:
                            nc.vector.scalar_tensor_tensor(
                                out=prod_f,
                                in0=ps,
                                scalar=1.0,
                                in1=prod_f,
                                op0=mybir.AluOpType.add,
                                op1=mybir.AluOpType.mult,
                            )
                    else:
                        tmp = tmp_pool.tile([128, 4 * N2], bf16)
                        nc.scalar.activation(tmp, ps, AF.Copy, bias=1.0)
                        if path == "D":
                            if first["D"]:
                                nc.vector.tensor_copy(prod_b, tmp)
                                first["D"] = False
                            else:
                                nc.vector.tensor_mul(prod_b, tmp, prod_b)
                        else:
                            if first["G"]:
                                nc.gpsimd.tensor_copy(prod_g, tmp)
                                first["G"] = False
                            else:
                                nc.gpsimd.tensor_mul(prod_g, tmp, prod_g)

            ln_f = persist.tile([128, 4 * N2], f32)
            nc.scalar.activation(ln_f, prod_f, AF.Ln)
            ln_b = persist.tile([128, 4 * N2], f32)
            nc.scalar.activation(ln_b, prod_b, AF.Ln)
            ln_g = persist.tile([128, 4 * N2], f32)
            nc.scalar.activation(ln_g, prod_g, AF.Ln)
            nc.sync.dma_start(out=out[0], in_=ln_f)
            nc.sync.dma_start(out=out[1], in_=ln_b)
            nc.sync.dma_start(out=out[2], in_=ln_g)

    nc.compile()
    return nc


def _get_nc():
    if "nc" not in _CACHE:
        _CACHE["nc"] = _build_nc()
    return _CACHE["nc"]


def _prep_core_inputs(loc_np: np.ndarray, mc_np: np.ndarray):
    """Shard/arrange host data into per-core input dicts."""
    import ml_dtypes

    bf16 = ml_dtypes.bfloat16
    locT = np.ascontiguousarray(loc_np.T).astype(bf16)  # (128, 512)
    in_maps = []
    for c in range(N_CORES):
        sl = mc_np[c * S_PER_CORE : (c + 1) * S_PER_CORE]  # (125, 512, 128)
        # per sample we need mc[n]^T = (d=128, j=512); group DMA_GROUP samples
        # along the free axis:  (25, 5, 512, 128) -> (25, 128, 5, 512)
        mct = sl.reshape(N_GROUPS, DMA_GROUP, N2, D).transpose(0, 3, 1, 2)
        mct = np.ascontiguousarray(mct).reshape(N_GROUPS, 128, DMA_GROUP * N2)
        in_maps.append({"locT": locT, "mc": mct.astype(bf16)})
    return in_maps


def _run_device(in_maps):
    from concourse import bass_utils

    nc = _get_nc()
    trace = bool(int(os.environ.get("BASSKL_TRACE", "0")))
    res = bass_utils.run_bass_kernel_spmd(
        nc, in_maps, core_ids=list(range(N_CORES)), trace=trace
    )
    LAST_INFO["exec_time_ns"] = res.exec_time_ns
    LAST_INFO["profile_json"] = res.profile_json
    return res.results


def _contrastive(mean_log: np.ndarray, scale_np: np.ndarray) -> float:
    """The tiny (2n,2n) tail of the loss, in f64 on host."""
    from scipy.special import digamma, gammaln

    scale = scale_np.astype(np.float64)
    b = (D - 1) / 2.0
    a = b + scale
    logZ = -((a + b) * np.log(2.0) + gammaln(a) - gammaln(a + b) + b * np.log(np.pi))
    H_p = -(logZ + scale * (np.log(2.0) + digamma(a) - digamma(a + b)))
    E_q = logZ[None, :] + scale[None, :] * mean_log
    sim = -H_p[None, :] - E_q
    idx = np.arange(N2)
    sim[idx, idx] = DIAG_FILL
    sim = sim * TEMPERATURE
    pos = sim[idx, (idx - N2 // 2) % N2]
    mx = sim.max(axis=-1)
    lse = mx + np.log(np.exp(sim - mx[:, None]).sum(-1))
    return float((pos - lse).mean())


def kernel(loc1, scale1, loc2, scale2):
    loc = np.concatenate([np.asarray(loc1), np.asarray(loc2)], axis=0).astype(
        np.float32
    )
    scale = np.concatenate([np.asarray(scale1), np.asarray(scale2)], axis=0).astype(
        np.float32
    )

    mc = _host_samples(loc, scale)            # (1000, 512, 128) f32
    in_maps = _prep_core_inputs(loc, mc)
    results = _run_device(in_maps)

    # gather: out is (2, 128, 2048): [ln_f | ln_b], blocks of i along free axis
    S = np.zeros((N2, N2), np.float64)
    for c in range(N_CORES):
        o = results[c]["out"].astype(np.float64)  # (3, 128, 2048)
        both = o[0] + o[1] + o[2]                 # (128, 2048)
        for blk in range(4):
            S[blk * 128 : (blk + 1) * 128, :] += both[:, blk * N2 : (blk + 1) * N2]
    mean_log = S / float(N_MC)

    loss = _contrastive(mean_log, scale)
    return np.float32(loss)



# revision 2
# speedup vs baseline: 1.0742x; 1.0742x over previous
"""Trainium2 Bass kernel for nn_KL_PS_Loss (PowerSpherical contrastive KL loss).

Host side: replicates the reference's MC sampling (jax CPU, key 42) exactly,
shards the n_mc=1000 sample axis across 8 NeuronCores.
Device side (per core, 125 samples): m = loc @ mc_n^T via bf16 matmuls into
f32 PSUM, then a running elementwise product of (1+m) — split between a
DVE scalar_tensor_tensor path (f32, fused (psum+1)*prod, 1x) and an
ACT-evacuate path (Identity+bias -> bf16 SBUF, then DVE tensor_mul at 2x) to
balance ACT/DVE load.  ln(prod) once at the end recovers sum(log1p(m)).
Host gathers the 8 partial sums, means, and runs the tiny (512,512)
contrastive step in f64.
"""

import os
import numpy as np

# ---- problem constants (hardcoded; must match reference.py) ----
N_MC = 1000
N2, D = 512, 128          # 2n, d
TEMPERATURE = 0.1
DIAG_FILL = -9e15
_EPS = 1e-12
N_CORES = 8
S_PER_CORE = N_MC // N_CORES   # 125
DMA_GROUP = 5                  # samples per input DMA
N_GROUPS = S_PER_CORE // DMA_GROUP  # 25
STT_EVERY = 4                  # kept for test_sim compatibility
# 6S/5D/5G per 16 samples; D-path uses ping-pong buffers so the DVE bf16
# multiply is not in-place (in-place blocks the 2x_1P perf mode).
PATTERN = ["S", "D", "G"] * 5 + ["S"]

_CACHE = {}
LAST_INFO = {}


def _host_samples(loc_np: np.ndarray, scale_np: np.ndarray) -> np.ndarray:
    """Exact replica of reference._rsample(jax.random.key(42), loc, scale, N_MC)
    on the jax CPU backend.  Returns (N_MC, 2n, d) float32."""
    import jax
    import jax.numpy as jnp

    cpu = jax.devices("cpu")[0]
    with jax.default_device(cpu):
        loc = jnp.asarray(loc_np, jnp.float32)
        scale = jnp.asarray(scale_np, jnp.float32)
        n, d = loc.shape
        b = (d - 1) / 2.0
        a = b + scale
        key = jax.random.key(42)
        k1, k2 = jax.random.split(key)
        z = jax.random.beta(k1, a, b, shape=(N_MC, n))
        t = 2.0 * z - 1.0
        v = jax.random.normal(k2, (N_MC, n, d - 1))
        v = v / jnp.linalg.norm(v, axis=-1, keepdims=True)
        t_ = t[..., None]
        y = jnp.concatenate(
            [t_, v * jnp.sqrt(jnp.clip(1.0 - t_ * t_, 1e-20))], axis=-1
        )
        e1 = jnp.zeros((n, d), loc.dtype).at[:, 0].set(1.0)
        u = e1 - loc
        u = u / (jnp.linalg.norm(u, axis=-1, keepdims=True) + _EPS)
        mc = y - 2.0 * jnp.sum(y * u, axis=-1, keepdims=True) * u
        return np.asarray(mc, dtype=np.float32)


def _build_nc():
    """Build the per-core Bass/Tile program (SPMD; same NEFF on all cores)."""
    import concourse.tile as tile
    from concourse import bacc, mybir

    f32 = mybir.dt.float32
    bf16 = mybir.dt.bfloat16
    AF = mybir.ActivationFunctionType

    nc = bacc.Bacc()
    locT = nc.dram_tensor("locT", (128, N2), bf16, kind="ExternalInput")
    mc = nc.dram_tensor(
        "mc", (N_GROUPS, 128, DMA_GROUP * N2), bf16, kind="ExternalInput"
    )
    out = nc.dram_tensor("out", (3, 128, 4 * N2), f32, kind="ExternalOutput")

    with tile.TileContext(nc) as tc:
        with (
            tc.tile_pool(name="const", bufs=1) as const_pool,
            tc.tile_pool(name="mcp", bufs=3) as mc_pool,
            tc.tile_pool(name="tmpp", bufs=4) as tmp_pool,
            tc.tile_pool(name="persist", bufs=1) as persist,
            tc.tile_pool(name="ps", bufs=2, space="PSUM") as ps_pool,
        ):
            locT_sb = const_pool.tile([128, N2], bf16)
            nc.sync.dma_start(out=locT_sb, in_=locT[:, :])

            # running products: prod_f (f32, fused-STT on DVE straight from
            # PSUM), prod_b (bf16, DVE mult), prod_g (bf16, GPSIMD mult).
            prod_f = persist.tile([128, 4 * N2], f32)
            prod_b0 = persist.tile([128, 4 * N2], bf16)
            prod_b1 = persist.tile([128, 4 * N2], bf16)
            prod_bs = [prod_b0, prod_b1]
            prod_g = persist.tile([128, 4 * N2], bf16)

            first = {"D": True, "G": True}
            dstate = {"cur": 0, "n": 0}
            for g in range(N_GROUPS):
                mc_sb = mc_pool.tile([128, DMA_GROUP * N2], bf16)
                nc.sync.dma_start(out=mc_sb, in_=mc[g])
                for r in range(DMA_GROUP):
                    s = g * DMA_GROUP + r
                    rhs = mc_sb[:, r * N2 : (r + 1) * N2]
                    ps = ps_pool.tile([128, 4 * N2], f32)
                    for blk in range(4):
                        nc.tensor.matmul(
                            ps[:, blk * N2 : (blk + 1) * N2],
                            locT_sb[:, blk * 128 : (blk + 1) * 128],
                            rhs,
                            start=True,
                            stop=True,
                        )
                    path = PATTERN[s % len(PATTERN)]
                    if path == "S":
                        if s == 0:
                            nc.vector.tensor_scalar_add(prod_f, ps, 1.0)
                        else:
                            nc.vector.scalar_tensor_tensor(
                                out=prod_f,
                                in0=ps,
                                scalar=1.0,
                                in1=prod_f,
                                op0=mybir.AluOpType.add,
                                op1=mybir.AluOpType.mult,
                            )
                    else:
                        tmp = tmp_pool.tile([128, 4 * N2], bf16)
                        nc.scalar.activation(tmp, ps, AF.Copy, bias=1.0)
                        if path == "D":
                            if first["D"]:
                                nc.vector.tensor_copy(prod_bs[0], tmp)
                                first["D"] = False
                            else:
                                cur = dstate["cur"]
                                nc.vector.tensor_mul(
                                    prod_bs[1 - cur], tmp, prod_bs[cur]
                                )
                                dstate["cur"] = 1 - cur
                            dstate["n"] += 1
                        else:
                            if first["G"]:
                                nc.gpsimd.tensor_copy(prod_g, tmp)
                                first["G"] = False
                            else:
                                nc.gpsimd.tensor_mul(prod_g, tmp, prod_g)

            ln_f = persist.tile([128, 4 * N2], f32)
            nc.scalar.activation(ln_f, prod_f, AF.Ln)
            ln_b = persist.tile([128, 4 * N2], f32)
            nc.scalar.activation(ln_b, prod_bs[dstate["cur"]], AF.Ln)
            ln_g = persist.tile([128, 4 * N2], f32)
            nc.scalar.activation(ln_g, prod_g, AF.Ln)
            nc.sync.dma_start(out=out[0], in_=ln_f)
            nc.sync.dma_start(out=out[1], in_=ln_b)
            nc.sync.dma_start(out=out[2], in_=ln_g)

    nc.compile()
    return nc


def _get_nc():
    if "nc" not in _CACHE:
        _CACHE["nc"] = _build_nc()
    return _CACHE["nc"]


def _prep_core_inputs(loc_np: np.ndarray, mc_np: np.ndarray):
    """Shard/arrange host data into per-core input dicts."""
    import ml_dtypes

    bf16 = ml_dtypes.bfloat16
    locT = np.ascontiguousarray(loc_np.T).astype(bf16)  # (128, 512)
    in_maps = []
    for c in range(N_CORES):
        sl = mc_np[c * S_PER_CORE : (c + 1) * S_PER_CORE]  # (125, 512, 128)
        # per sample we need mc[n]^T = (d=128, j=512); group DMA_GROUP samples
        # along the free axis:  (25, 5, 512, 128) -> (25, 128, 5, 512)
        mct = sl.reshape(N_GROUPS, DMA_GROUP, N2, D).transpose(0, 3, 1, 2)
        mct = np.ascontiguousarray(mct).reshape(N_GROUPS, 128, DMA_GROUP * N2)
        in_maps.append({"locT": locT, "mc": mct.astype(bf16)})
    return in_maps


def _run_device(in_maps):
    from concourse import bass_utils

    nc = _get_nc()
    trace = bool(int(os.environ.get("BASSKL_TRACE", "0")))
    res = bass_utils.run_bass_kernel_spmd(
        nc, in_maps, core_ids=list(range(N_CORES)), trace=trace
    )
    LAST_INFO["exec_time_ns"] = res.exec_time_ns
    LAST_INFO["profile_json"] = res.profile_json
    return res.results


def _contrastive(mean_log: np.ndarray, scale_np: np.ndarray) -> float:
    """The tiny (2n,2n) tail of the loss, in f64 on host."""
    from scipy.special import digamma, gammaln

    scale = scale_np.astype(np.float64)
    b = (D - 1) / 2.0
    a = b + scale
    logZ = -((a + b) * np.log(2.0) + gammaln(a) - gammaln(a + b) + b * np.log(np.pi))
    H_p = -(logZ + scale * (np.log(2.0) + digamma(a) - digamma(a + b)))
    E_q = logZ[None, :] + scale[None, :] * mean_log
    sim = -H_p[None, :] - E_q
    idx = np.arange(N2)
    sim[idx, idx] = DIAG_FILL
    sim = sim * TEMPERATURE
    pos = sim[idx, (idx - N2 // 2) % N2]
    mx = sim.max(axis=-1)
    lse = mx + np.log(np.exp(sim - mx[:, None]).sum(-1))
    return float((pos - lse).mean())


def kernel(loc1, scale1, loc2, scale2):
    loc = np.concatenate([np.asarray(loc1), np.asarray(loc2)], axis=0).astype(
        np.float32
    )
    scale = np.concatenate([np.asarray(scale1), np.asarray(scale2)], axis=0).astype(
        np.float32
    )

    mc = _host_samples(loc, scale)            # (1000, 512, 128) f32
    in_maps = _prep_core_inputs(loc, mc)
    results = _run_device(in_maps)

    # gather: out is (2, 128, 2048): [ln_f | ln_b], blocks of i along free axis
    S = np.zeros((N2, N2), np.float64)
    for c in range(N_CORES):
        o = results[c]["out"].astype(np.float64)  # (3, 128, 2048)
        both = o[0] + o[1] + o[2]                 # (128, 2048)
        for blk in range(4):
            S[blk * 128 : (blk + 1) * 128, :] += both[:, blk * N2 : (blk + 1) * N2]
    mean_log = S / float(N_MC)

    loss = _contrastive(mean_log, scale)
    return np.float32(loss)



# revision 3
# speedup vs baseline: 1.1291x; 1.0510x over previous
"""Trainium2 Bass kernel for nn_KL_PS_Loss (PowerSpherical contrastive KL loss).

Host side: replicates the reference's MC sampling (jax CPU, key 42) exactly,
shards the n_mc=1000 sample axis across 8 NeuronCores.
Device side (per core, 125 samples): m = loc @ mc_n^T via bf16 matmuls into
f32 PSUM, then a running elementwise product of (1+m) — split between a
DVE scalar_tensor_tensor path (f32, fused (psum+1)*prod, 1x) and an
ACT-evacuate path (Identity+bias -> bf16 SBUF, then DVE tensor_mul at 2x) to
balance ACT/DVE load.  ln(prod) once at the end recovers sum(log1p(m)).
Host gathers the 8 partial sums, means, and runs the tiny (512,512)
contrastive step in f64.
"""

import os
import numpy as np

# ---- problem constants (hardcoded; must match reference.py) ----
N_MC = 1000
N2, D = 512, 128          # 2n, d
TEMPERATURE = 0.1
DIAG_FILL = -9e15
_EPS = 1e-12
N_CORES = 8
S_PER_CORE = N_MC // N_CORES   # 125
DMA_GROUP = 5                  # samples per input DMA
N_GROUPS = S_PER_CORE // DMA_GROUP  # 25
STT_EVERY = 4                  # kept for test_sim compatibility
# 6S/5D/5G per 16 samples; D-path uses ping-pong buffers so the DVE bf16
# multiply is not in-place (in-place blocks the 2x_1P perf mode).
PATTERN = ["S", "D", "G"] * 5 + ["S"]

_CACHE = {}
LAST_INFO = {}


def _host_samples(loc_np: np.ndarray, scale_np: np.ndarray) -> np.ndarray:
    """Exact replica of reference._rsample(jax.random.key(42), loc, scale, N_MC)
    on the jax CPU backend.  Returns (N_MC, 2n, d) float32."""
    import jax
    import jax.numpy as jnp

    cpu = jax.devices("cpu")[0]
    with jax.default_device(cpu):
        loc = jnp.asarray(loc_np, jnp.float32)
        scale = jnp.asarray(scale_np, jnp.float32)
        n, d = loc.shape
        b = (d - 1) / 2.0
        a = b + scale
        key = jax.random.key(42)
        k1, k2 = jax.random.split(key)
        z = jax.random.beta(k1, a, b, shape=(N_MC, n))
        t = 2.0 * z - 1.0
        v = jax.random.normal(k2, (N_MC, n, d - 1))
        v = v / jnp.linalg.norm(v, axis=-1, keepdims=True)
        t_ = t[..., None]
        y = jnp.concatenate(
            [t_, v * jnp.sqrt(jnp.clip(1.0 - t_ * t_, 1e-20))], axis=-1
        )
        e1 = jnp.zeros((n, d), loc.dtype).at[:, 0].set(1.0)
        u = e1 - loc
        u = u / (jnp.linalg.norm(u, axis=-1, keepdims=True) + _EPS)
        mc = y - 2.0 * jnp.sum(y * u, axis=-1, keepdims=True) * u
        return np.asarray(mc, dtype=np.float32)


def _build_nc():
    """Build the per-core Bass/Tile program (SPMD; same NEFF on all cores)."""
    import concourse.tile as tile
    from concourse import bacc, mybir

    f32 = mybir.dt.float32
    bf16 = mybir.dt.bfloat16
    AF = mybir.ActivationFunctionType

    nc = bacc.Bacc()
    locT = nc.dram_tensor("locT", (128, N2), bf16, kind="ExternalInput")
    mc = nc.dram_tensor(
        "mc", (N_GROUPS, 128, DMA_GROUP * N2), bf16, kind="ExternalInput"
    )
    out = nc.dram_tensor("out", (3, 128, 4 * N2), f32, kind="ExternalOutput")

    with tile.TileContext(nc) as tc:
        with (
            tc.tile_pool(name="const", bufs=1) as const_pool,
            tc.tile_pool(name="mcp", bufs=3) as mc_pool,
            tc.tile_pool(name="tmpp", bufs=6) as tmp_pool,
            tc.tile_pool(name="persist", bufs=1) as persist,
            tc.tile_pool(name="ps", bufs=4, space="PSUM") as ps_pool,
        ):
            locT_sb = const_pool.tile([128, N2], bf16)
            nc.sync.dma_start(out=locT_sb, in_=locT[:, :])

            # running products: prod_f (f32, fused-STT on DVE straight from
            # PSUM), prod_b (bf16, DVE mult), prod_g (bf16, GPSIMD mult).
            prod_f = persist.tile([128, 4 * N2], f32)
            prod_b0 = persist.tile([128, 4 * N2], bf16)
            prod_b1 = persist.tile([128, 4 * N2], bf16)
            prod_bs = [prod_b0, prod_b1]
            prod_g = persist.tile([128, 4 * N2], bf16)

            first = {"D": True, "G": True}
            dstate = {"cur": 0, "n": 0}
            # Deferred mult ops: (sample, engine_op_closure). Emitting the
            # D/G-path multiplies ~2 samples late keeps them from head-of-line
            # blocking the strict-FIFO DVE/GPSIMD queues while ACT evacuates.
            pending = []

            def flush_pending(upto):
                while pending and pending[0][0] <= upto:
                    pending.pop(0)[1]()
            for g in range(N_GROUPS):
                mc_sb = mc_pool.tile([128, DMA_GROUP * N2], bf16)
                nc.sync.dma_start(out=mc_sb, in_=mc[g])
                for r in range(DMA_GROUP):
                    s = g * DMA_GROUP + r
                    flush_pending(s - 2)
                    rhs = mc_sb[:, r * N2 : (r + 1) * N2]
                    # two half-width psum tiles per sample: 4-slot rotation
                    # decouples PE refills from the single consumer per tile
                    psh = [
                        ps_pool.tile([128, 2 * N2], f32, name="psh")
                        for _ in range(2)
                    ]
                    for blk in range(4):
                        nc.tensor.matmul(
                            psh[blk // 2][:, (blk % 2) * N2 : (blk % 2 + 1) * N2],
                            locT_sb[:, blk * 128 : (blk + 1) * 128],
                            rhs,
                            start=True,
                            stop=True,
                        )
                    path = PATTERN[s % len(PATTERN)]
                    if path == "S":
                        for h in range(2):
                            sl = prod_f[:, h * 2 * N2 : (h + 1) * 2 * N2]
                            if s == 0:
                                nc.vector.tensor_scalar_add(sl, psh[h], 1.0)
                            else:
                                nc.vector.scalar_tensor_tensor(
                                    out=sl,
                                    in0=psh[h],
                                    scalar=1.0,
                                    in1=sl,
                                    op0=mybir.AluOpType.add,
                                    op1=mybir.AluOpType.mult,
                                )
                    else:
                        tmp = tmp_pool.tile([128, 4 * N2], bf16)
                        for h in range(2):
                            nc.scalar.activation(
                                tmp[:, h * 2 * N2 : (h + 1) * 2 * N2],
                                psh[h],
                                AF.Copy,
                                bias=1.0,
                            )
                        if path == "D":

                            def dmul(tmp=tmp):
                                if first["D"]:
                                    nc.vector.tensor_copy(prod_bs[0], tmp)
                                    first["D"] = False
                                else:
                                    cur = dstate["cur"]
                                    nc.vector.tensor_mul(
                                        prod_bs[1 - cur], tmp, prod_bs[cur]
                                    )
                                    dstate["cur"] = 1 - cur
                                dstate["n"] += 1

                            pending.append((s, dmul))
                        else:

                            def gmul(tmp=tmp):
                                if first["G"]:
                                    nc.gpsimd.tensor_copy(prod_g, tmp)
                                    first["G"] = False
                                else:
                                    nc.gpsimd.tensor_mul(prod_g, tmp, prod_g)

                            pending.append((s, gmul))

            flush_pending(S_PER_CORE)

            ln_f = persist.tile([128, 4 * N2], f32)
            nc.scalar.activation(ln_f, prod_f, AF.Ln)
            ln_b = persist.tile([128, 4 * N2], f32)
            nc.scalar.activation(ln_b, prod_bs[dstate["cur"]], AF.Ln)
            ln_g = persist.tile([128, 4 * N2], f32)
            nc.scalar.activation(ln_g, prod_g, AF.Ln)
            nc.sync.dma_start(out=out[0], in_=ln_f)
            nc.sync.dma_start(out=out[1], in_=ln_b)
            nc.sync.dma_start(out=out[2], in_=ln_g)

    nc.compile()
    return nc


def _get_nc():
    if "nc" not in _CACHE:
        _CACHE["nc"] = _build_nc()
    return _CACHE["nc"]


def _prep_core_inputs(loc_np: np.ndarray, mc_np: np.ndarray):
    """Shard/arrange host data into per-core input dicts."""
    import ml_dtypes

    bf16 = ml_dtypes.bfloat16
    locT = np.ascontiguousarray(loc_np.T).astype(bf16)  # (128, 512)
    in_maps = []
    for c in range(N_CORES):
        sl = mc_np[c * S_PER_CORE : (c + 1) * S_PER_CORE]  # (125, 512, 128)
        # per sample we need mc[n]^T = (d=128, j=512); group DMA_GROUP samples
        # along the free axis:  (25, 5, 512, 128) -> (25, 128, 5, 512)
        mct = sl.reshape(N_GROUPS, DMA_GROUP, N2, D).transpose(0, 3, 1, 2)
        mct = np.ascontiguousarray(mct).reshape(N_GROUPS, 128, DMA_GROUP * N2)
        in_maps.append({"locT": locT, "mc": mct.astype(bf16)})
    return in_maps


def _run_device(in_maps):
    from concourse import bass_utils

    nc = _get_nc()
    trace = bool(int(os.environ.get("BASSKL_TRACE", "0")))
    res = bass_utils.run_bass_kernel_spmd(
        nc, in_maps, core_ids=list(range(N_CORES)), trace=trace
    )
    LAST_INFO["exec_time_ns"] = res.exec_time_ns
    LAST_INFO["profile_json"] = res.profile_json
    return res.results


def _contrastive(mean_log: np.ndarray, scale_np: np.ndarray) -> float:
    """The tiny (2n,2n) tail of the loss, in f64 on host."""
    from scipy.special import digamma, gammaln

    scale = scale_np.astype(np.float64)
    b = (D - 1) / 2.0
    a = b + scale
    logZ = -((a + b) * np.log(2.0) + gammaln(a) - gammaln(a + b) + b * np.log(np.pi))
    H_p = -(logZ + scale * (np.log(2.0) + digamma(a) - digamma(a + b)))
    E_q = logZ[None, :] + scale[None, :] * mean_log
    sim = -H_p[None, :] - E_q
    idx = np.arange(N2)
    sim[idx, idx] = DIAG_FILL
    sim = sim * TEMPERATURE
    pos = sim[idx, (idx - N2 // 2) % N2]
    mx = sim.max(axis=-1)
    lse = mx + np.log(np.exp(sim - mx[:, None]).sum(-1))
    return float((pos - lse).mean())


def kernel(loc1, scale1, loc2, scale2):
    loc = np.concatenate([np.asarray(loc1), np.asarray(loc2)], axis=0).astype(
        np.float32
    )
    scale = np.concatenate([np.asarray(scale1), np.asarray(scale2)], axis=0).astype(
        np.float32
    )

    mc = _host_samples(loc, scale)            # (1000, 512, 128) f32
    in_maps = _prep_core_inputs(loc, mc)
    results = _run_device(in_maps)

    # gather: out is (2, 128, 2048): [ln_f | ln_b], blocks of i along free axis
    S = np.zeros((N2, N2), np.float64)
    for c in range(N_CORES):
        o = results[c]["out"].astype(np.float64)  # (3, 128, 2048)
        both = o[0] + o[1] + o[2]                 # (128, 2048)
        for blk in range(4):
            S[blk * 128 : (blk + 1) * 128, :] += both[:, blk * N2 : (blk + 1) * N2]
    mean_log = S / float(N_MC)

    loss = _contrastive(mean_log, scale)
    return np.float32(loss)

